# revision 20
# baseline (speedup 1.0000x reference)
"""BERT self-attention (B=4, S=1024, HID=1024, NH=16, HD=64) on 8 TRN2 NeuronCores.

Sharding: 8 shards = 4 batches x 2 head-halves. Core c handles batch c%4 and
heads [g*8, g*8+8) with g = c//4. Each core computes q/k/v projections for its
512 feature columns and full attention for its 8 heads; no collectives needed.
The host pre-transposes hidden_states / weights so the device never transposes.

Device-side layout choices:
  - q^T, k^T kept as [feat, seq] (feat on partitions): scores are computed
    TRANSPOSED, s^T[keys, queries] = k^T.T @ q^T, so softmax's exp needs no
    reduction first and the attention-mask bias is a per-partition ACT bias.
  - exp(s/8 + maskbias) goes straight from PSUM through the scalar engine into
    SBUF as unnormalized probabilities p~^T [keys, queries]; two key-chunks are
    paired per ACT op (N=1024) to amortize the ~352-cycle ACT fixed cost.
  - v is produced as [seq, feat] with a ones-column appended per head
    (v_aug [seq, 65]); ctx~^T = v_aug.T @ p~^T and row 64 of the PSUM result is
    the softmax denominator. Normalize: DVE reciprocal + gpsimd partition
    broadcast + DVE multiply.
  - all matmuls run as float32r (fp32 data, single-pass PE mode, ~1e-4 rel).
  - emission interleaves q/k projection chunks into the attention loop so the
    PE stays busy (and HAM-warm) while the scalar engine chews through exp.
Host reassembles: out[h] is ctx^T [64, 1024] -> transpose -> output columns.
"""
import os
import sys
from contextlib import ExitStack

for _p in ("/root/.axon_site/_ro/trn_rl_repo", "/opt/trn_rl_repo"):
    if os.path.isdir(_p) and _p not in sys.path:
        sys.path.append(_p)

import numpy as np
import concourse.bacc as bacc
import concourse.mybir as mybir
from concourse import tile
from concourse.bass_utils import run_bass_kernel_spmd

B, S, HID, NH, HD = 4, 1024, 1024, 16, 64
NCORES = 8
FSH = 512  # feature columns per core = 8 heads * 64
HC = 8  # hid contraction chunks of 128
JC = 8  # key/seq chunks of 128
SC = 2  # seq chunks of 512 (queries / moving dim)
FC = 4  # feature chunks of 128
NHL = 8  # local heads per core

F32 = mybir.dt.float32
F32R = mybir.dt.float32r
F16 = mybir.dt.float16
EXP = mybir.ActivationFunctionType.Exp


def _r(ap):
    return ap.bitcast(F32R)


def _build_nc():
    nc = bacc.Bacc(None, target_bir_lowering=False, debug=False)

    hsT = nc.declare_dram_parameter("hsT", [128, HC, S], F16, isOutput=False)
    wqT = nc.declare_dram_parameter("wqT", [128, HC, FSH], F16, isOutput=False)
    wkT = nc.declare_dram_parameter("wkT", [128, HC, FSH], F16, isOutput=False)
    wvT = nc.declare_dram_parameter("wvT", [128, HC, FSH], F16, isOutput=False)
    bqc = nc.declare_dram_parameter("bqc", [128, FC], F32, isOutput=False)
    bkc = nc.declare_dram_parameter("bkc", [128, FC], F32, isOutput=False)
    bv1 = nc.declare_dram_parameter("bv1", [1, FSH], F32, isOutput=False)
    mb = nc.declare_dram_parameter("mb", [128, JC], F32, isOutput=False)
    out = nc.declare_dram_parameter("out", [NHL, HD, S], F32, isOutput=True)

    with tile.TileContext(nc) as tc, ExitStack() as ctx:
        ctx.enter_context(
            nc.allow_low_precision(reason="float32r is 4-byte storage; PE fast path")
        )
        const = ctx.enter_context(tc.tile_pool(name="const", bufs=1))
        ps_p = ctx.enter_context(tc.tile_pool(name="ps_p", bufs=2, space="PSUM"))
        p_pool = ctx.enter_context(tc.tile_pool(name="p", bufs=2))
        sm = ctx.enter_context(tc.tile_pool(name="sm", bufs=2))

        hsT_sb = const.tile([128, HC, S], F16, tag="hsT")
        wq_sb = const.tile([128, HC, FSH], F16, tag="wq")
        wk_sb = const.tile([128, HC, FSH], F16, tag="wk")
        wv_sb = const.tile([128, HC, FSH], F16, tag="wv")
        # two HWDGE rings (sync + scalar-engine) drain inputs in parallel;
        # ring A carries what the first qk0 matmuls need soonest
        HH = HC // 2
        h1, h2 = slice(0, HH), slice(HH, HC)
        nc.sync.dma_start(hsT_sb[:, h1, :], hsT[:, h1, :])
        nc.scalar.dma_start(wq_sb[:, h1, :], wqT[:, h1, :])
        nc.scalar.dma_start(wk_sb[:, h1, :], wkT[:, h1, :])
        nc.sync.dma_start(hsT_sb[:, h2, :], hsT[:, h2, :])
        nc.scalar.dma_start(wq_sb[:, h2, :], wqT[:, h2, :])
        nc.scalar.dma_start(wk_sb[:, h2, :], wkT[:, h2, :])
        nc.sync.dma_start(wv_sb[:], wvT[:])
        bq_sb = const.tile([128, FC], F32, tag="bq")
        bk_sb = const.tile([128, FC], F32, tag="bk")
        bv_sb = const.tile([1, FSH], F32R, tag="bv")
        mb_sb = const.tile([128, JC], F32, tag="mb")
        nc.sync.dma_start(bq_sb[:], bqc[:])
        nc.sync.dma_start(bk_sb[:], bkc[:])
        nc.sync.dma_start(bv_sb[:], _r(bv1[:]))
        nc.sync.dma_start(mb_sb[:], mb[:])
        ones_sb = const.tile([1, 128], F32R, tag="ones")
        nc.vector.memset(ones_sb[:].bitcast(F32), 1.0)

        qT_sb = const.tile([128, FC, S], F32R, tag="qT")
        kT_sb = const.tile([128, FC, S], F32R, tag="kT")
        # v with per-head ones column: [seq_part, jc, head, 64 v + 1 one]
        v_sb = const.tile([128, JC, NHL, HD + 1], F32R, tag="v")
        nc.vector.memset(v_sb[:].bitcast(F32), 1.0)

        def emit_qk_proj(fc, which=None):
            """q^T,k^T projection for feature chunk fc (pack fc's 2 heads).
            which=0 emits only q, which=1 only k, None both."""
            parts = ((wq_sb, bq_sb, qT_sb), (wk_sb, bk_sb, kT_sb))
            if which is not None:
                parts = (parts[which],)
            for w_sb, b_sb, dst in parts:
                for sc in range(SC):
                    ps = ps_p.tile([128, 512], F32, tag="pp", name=f"pp{fc}{sc}")
                    for hc in range(HC):
                        nc.tensor.matmul(
                            ps[:],
                            w_sb[:, hc, fc * 128 : (fc + 1) * 128],
                            hsT_sb[:, hc, sc * 512 : (sc + 1) * 512],
                            start=(hc == 0),
                            stop=(hc == HC - 1),
                        )
                    nc.vector.tensor_scalar_add(
                        dst[:, fc, sc * 512 : (sc + 1) * 512],
                        ps[:],
                        b_sb[:, fc : fc + 1],
                    )

        def emit_v_proj():
            """v projection [seq, feat] + bias via ones-matmul."""
            for jc in range(JC):
                ps = ps_p.tile([128, 512], F32, tag="pp", name=f"ppv{jc}")
                for hc in range(HC):
                    nc.tensor.matmul(
                        ps[:],
                        hsT_sb[:, hc, jc * 128 : (jc + 1) * 128],
                        wv_sb[:, hc, :],
                        start=(hc == 0),
                        stop=False,
                    )
                nc.tensor.matmul(
                    ps[:], ones_sb[:, 0:128], bv_sb[:], start=False, stop=True
                )
                nc.vector.tensor_copy(
                    v_sb[:, jc, :, 0:HD], ps[:].rearrange("p (h d) -> p h d", h=NHL)
                )

        def emit_scores(g2, i, ptb):
            """Scores + exp for pack g2, query chunk i. The two heads are
            row-tiled concurrently on the PE (K=64 each) and share one N=1024
            ACT exp (same key-chunk -> same mask bias, exact for any mask)."""
            for jc in range(JC):
                ps = ps_s.tile([128, 1024], F32, tag="ss", name=f"ss{jc}")
                for hh in range(2):
                    lo = hh * 64
                    nc.tensor.matmul(
                        ps[:, hh * 512 : (hh + 1) * 512],
                        kT_sb[lo : lo + 64, g2, jc * 128 : (jc + 1) * 128],
                        qT_sb[lo : lo + 64, g2, i * 512 : (i + 1) * 512],
                        start=True,
                        stop=True,
                        tile_position=(lo, 0),
                    )
                nc.scalar.activation(
                    ptb[:, :, jc, :],
                    ps[:].rearrange("p (a b) -> p a b", a=2),
                    EXP,
                    bias=mb_sb[:, jc : jc + 1],
                    scale=0.125,
                )

        def emit_ctx(g2, i, ptb):
            """ctx~^T + normalize + store for pack g2, query chunk i."""
            for hh in range(2):
                h = 2 * g2 + hh
                pc = ps_c.tile([HD + 1, 512], F32, tag="cc", name=f"cc{hh}")
                for jc in range(JC):
                    nc.tensor.matmul(
                        pc[:],
                        v_sb[:, jc, h, :],
                        ptb[:, hh, jc, :],
                        start=(jc == 0),
                        stop=(jc == JC - 1),
                    )
                den = sm.tile([1, 512], F32, tag="den", name=f"dn{hh}")
                nc.vector.tensor_copy(den[:], pc[HD : HD + 1, :])
                recip = sm.tile([1, 512], F32, tag="recip", name=f"rc{hh}")
                nc.vector.reciprocal_approx_fast(recip[:], den[:])
                pbs = sm.tile([64, 512], F32, tag="pbs", name=f"pb{hh}")
                nc.gpsimd.partition_broadcast(pbs[:], recip[0:1, :])
                ob = sm.tile([64, 512], F32, tag="ob", name=f"ob{hh}")
                nc.vector.tensor_mul(ob[:], pc[0:HD, :], pbs[:])
                nc.sync.dma_start(out[h, :, i * 512 : (i + 1) * 512], ob[:])

        # ---- emission schedule: keep PE dense while ACT drains exp ----
        # pack-0 q/k projections up front so scores start as soon as the
        # first weight chunks land; each iteration then gets dependency-free
        # projection matmuls as PE filler while ACT chews this pack's exp:
        #   iter0: v-projection (ctx(0,0) needs it at iter end)
        #   iter1: pack-1 q+k;  iter2/3: pack-2 q/k;  iter4/5: pack-3 q/k
        with tc.tile_pool(name="ps_p0", bufs=4, space="PSUM") as ps_p0:
            qk0 = []
            for w_sb, b_sb, dst in ((wq_sb, bq_sb, qT_sb), (wk_sb, bk_sb, kT_sb)):
                for sc in range(SC):
                    ps = ps_p0.tile([128, 512], F32, tag="pp0", name=f"p0{sc}")
                    qk0.append((ps, w_sb, b_sb, dst, sc))
            for half in range(2):
                for ps, w_sb, b_sb, dst, sc in qk0:
                    for hc in range(half * HH, (half + 1) * HH):
                        nc.tensor.matmul(
                            ps[:],
                            w_sb[:, hc, 0:128],
                            hsT_sb[:, hc, sc * 512 : (sc + 1) * 512],
                            start=(hc == 0),
                            stop=(hc == HC - 1),
                        )
            for ps, w_sb, b_sb, dst, sc in qk0:
                nc.vector.tensor_scalar_add(
                    dst[:, 0, sc * 512 : (sc + 1) * 512], ps[:], b_sb[:, 0:1]
                )
        ps_s = ctx.enter_context(tc.tile_pool(name="ps_s", bufs=2, space="PSUM"))
        ps_c = ctx.enter_context(tc.tile_pool(name="ps_c", bufs=2, space="PSUM"))
        fillers = [
            emit_v_proj,
            lambda: emit_qk_proj(1),
            lambda: emit_qk_proj(2, which=0),
            lambda: emit_qk_proj(2, which=1),
            lambda: emit_qk_proj(3, which=0),
            lambda: emit_qk_proj(3, which=1),
            None,
            None,
        ]
        step = 0
        for g2 in range(4):
            for i in range(SC):
                ptb = p_pool.tile([128, 2, JC, 512], F32R, tag="pt", name="ptb")
                emit_scores(g2, i, ptb)
                if fillers[step] is not None:
                    fillers[step]()
                step += 1
                emit_ctx(g2, i, ptb)

    nc.compile()
    return nc


_NC = None


def _get_nc():
    global _NC
    if _NC is None:
        _NC = _build_nc()
    return _NC


# test-harness knobs (ignored in normal grading use)
TRACE = False
TRACE_DIR = None
LAST_RESULT = None


def _pack(mT):
    """[1024, N] contraction-major -> [128, 8, N] partition-major fp16 so one
    DMA moves 8*N*2 contiguous bytes per partition (big DMA packets)."""
    n = mT.shape[1]
    return np.ascontiguousarray(
        mT.reshape(HC, 128, n).transpose(1, 0, 2)
    ).astype(np.float16)


def kernel(hidden_states, attention_mask, Wq, bq, Wk, bk, Wv, bv):
    global LAST_RESULT
    hs = np.asarray(hidden_states, dtype=np.float32)
    mask = np.asarray(attention_mask, dtype=np.float32)
    Wq = np.asarray(Wq, dtype=np.float32)
    Wk = np.asarray(Wk, dtype=np.float32)
    Wv = np.asarray(Wv, dtype=np.float32)
    bq = np.asarray(bq, dtype=np.float32)
    bk = np.asarray(bk, dtype=np.float32)
    bv = np.asarray(bv, dtype=np.float32)

    in_maps = []
    for c in range(NCORES):
        b, g = c % B, c // B
        sl = slice(g * FSH, (g + 1) * FSH)
        in_maps.append(
            {
                "hsT": _pack(hs[b].T),
                "wqT": _pack(Wq[sl, :].T),
                "wkT": _pack(Wk[sl, :].T),
                "wvT": _pack(Wv[sl, :].T),
                "bqc": np.ascontiguousarray(bq[sl].reshape(FC, 128).T),
                "bkc": np.ascontiguousarray(bk[sl].reshape(FC, 128).T),
                "bv1": np.ascontiguousarray(bv[sl].reshape(1, FSH)),
                "mb": np.ascontiguousarray(
                    ((mask[b, 0, 0, :] - 1.0) * 1.0e6).reshape(JC, 128).T
                ),
            }
        )

    nc = _get_nc()
    kw = {}
    if TRACE:
        kw = {"trace": True, "tmpdir": TRACE_DIR}
    res = run_bass_kernel_spmd(nc, in_maps, list(range(NCORES)), **kw)
    LAST_RESULT = res

    full = np.empty((B, S, HID), dtype=np.float32)
    for c in range(NCORES):
        b, g = c % B, c // B
        o = res.results[c]["out"]  # [NHL, HD, S]
        full[b, :, g * FSH : (g + 1) * FSH] = (
            o.transpose(2, 0, 1).reshape(S, FSH)
        )
    return full


# revision 23
# speedup vs baseline: 1.0727x; 1.0727x over previous
"""BERT self-attention (B=4, S=1024, HID=1024, NH=16, HD=64) on 8 TRN2 NeuronCores.

Sharding: 8 shards = 4 batches x 2 head-halves. Core c handles batch c%4 and
heads [g*8, g*8+8) with g = c//4. Each core computes q/k/v projections for its
512 feature columns and full attention for its 8 heads; no collectives needed.
The host pre-transposes hidden_states / weights so the device never transposes.

Device-side layout choices:
  - q^T, k^T kept as [feat, seq] (feat on partitions): scores are computed
    TRANSPOSED, s^T[keys, queries] = k^T.T @ q^T, so softmax's exp needs no
    reduction first and the attention-mask bias is a per-partition ACT bias.
  - exp(s/8 + maskbias) goes straight from PSUM through the scalar engine into
    SBUF as unnormalized probabilities p~^T [keys, queries]; two key-chunks are
    paired per ACT op (N=1024) to amortize the ~352-cycle ACT fixed cost.
  - v is produced as [seq, feat] with a ones-column appended per head
    (v_aug [seq, 65]); ctx~^T = v_aug.T @ p~^T and row 64 of the PSUM result is
    the softmax denominator. Normalize: DVE reciprocal + gpsimd partition
    broadcast + DVE multiply.
  - all matmuls run as float32r (fp32 data, single-pass PE mode, ~1e-4 rel).
  - emission interleaves q/k projection chunks into the attention loop so the
    PE stays busy (and HAM-warm) while the scalar engine chews through exp.
Host reassembles: out[h] is ctx^T [64, 1024] -> transpose -> output columns.
"""
import os
import sys
from contextlib import ExitStack

for _p in ("/root/.axon_site/_ro/trn_rl_repo", "/opt/trn_rl_repo"):
    if os.path.isdir(_p) and _p not in sys.path:
        sys.path.append(_p)

import numpy as np
import concourse.bacc as bacc
import concourse.mybir as mybir
from concourse import tile
from concourse.bass_utils import run_bass_kernel_spmd

B, S, HID, NH, HD = 4, 1024, 1024, 16, 64
NCORES = 8
FSH = 512  # feature columns per core = 8 heads * 64
HC = 8  # hid contraction chunks of 128
JC = 8  # key/seq chunks of 128
SC = 2  # seq chunks of 512 (queries / moving dim)
FC = 4  # feature chunks of 128
NHL = 8  # local heads per core

F32 = mybir.dt.float32
F32R = mybir.dt.float32r
F16 = mybir.dt.float16
EXP = mybir.ActivationFunctionType.Exp


def _r(ap):
    return ap.bitcast(F32R)


def _build_nc():
    nc = bacc.Bacc(None, target_bir_lowering=False, debug=False)

    hsT = nc.declare_dram_parameter("hsT", [128, HC, S], F16, isOutput=False)
    wqT = nc.declare_dram_parameter("wqT", [128, HC, FSH], F16, isOutput=False)
    wkT = nc.declare_dram_parameter("wkT", [128, HC, FSH], F16, isOutput=False)
    wvT = nc.declare_dram_parameter("wvT", [128, HC, FSH], F16, isOutput=False)
    bqc = nc.declare_dram_parameter("bqc", [128, FC], F32, isOutput=False)
    bkc = nc.declare_dram_parameter("bkc", [128, FC], F32, isOutput=False)
    bv1 = nc.declare_dram_parameter("bv1", [1, FSH], F32, isOutput=False)
    mb = nc.declare_dram_parameter("mb", [128, JC], F32, isOutput=False)
    out = nc.declare_dram_parameter("out", [NHL, HD, S], F32, isOutput=True)

    with tile.TileContext(nc) as tc, ExitStack() as ctx:
        ctx.enter_context(
            nc.allow_low_precision(reason="float32r is 4-byte storage; PE fast path")
        )
        const = ctx.enter_context(tc.tile_pool(name="const", bufs=1))
        ps_p = ctx.enter_context(tc.tile_pool(name="ps_p", bufs=2, space="PSUM"))
        p_pool = ctx.enter_context(tc.tile_pool(name="p", bufs=2))
        sm = ctx.enter_context(tc.tile_pool(name="sm", bufs=2))

        hsT_sb = const.tile([128, HC, S], F16, tag="hsT")
        wq_sb = const.tile([128, HC, FSH], F16, tag="wq")
        wk_sb = const.tile([128, HC, FSH], F16, tag="wk")
        wv_sb = const.tile([128, HC, FSH], F16, tag="wv")
        # two HWDGE rings (sync + scalar-engine) drain inputs in parallel;
        # ring A carries what the first qk0 matmuls need soonest
        HH = HC // 2
        h1, h2 = slice(0, HH), slice(HH, HC)
        nc.sync.dma_start(hsT_sb[:, h1, :], hsT[:, h1, :])
        nc.scalar.dma_start(wq_sb[:, h1, :], wqT[:, h1, :])
        nc.scalar.dma_start(wk_sb[:, h1, :], wkT[:, h1, :])
        nc.sync.dma_start(hsT_sb[:, h2, :], hsT[:, h2, :])
        nc.scalar.dma_start(wq_sb[:, h2, :], wqT[:, h2, :])
        nc.scalar.dma_start(wk_sb[:, h2, :], wkT[:, h2, :])
        nc.sync.dma_start(wv_sb[:], wvT[:])
        bq_sb = const.tile([128, FC], F32, tag="bq")
        bk_sb = const.tile([128, FC], F32, tag="bk")
        bv_sb = const.tile([1, FSH], F32R, tag="bv")
        mb_sb = const.tile([128, JC], F32, tag="mb")
        nc.sync.dma_start(bq_sb[:], bqc[:])
        nc.sync.dma_start(bk_sb[:], bkc[:])
        nc.sync.dma_start(bv_sb[:], _r(bv1[:]))
        nc.sync.dma_start(mb_sb[:], mb[:])
        ones_sb = const.tile([1, 128], F32R, tag="ones")
        nc.vector.memset(ones_sb[:].bitcast(F32), 1.0)

        qT_sb = const.tile([128, FC, S], F16, tag="qT")
        kT_sb = const.tile([128, FC, S], F16, tag="kT")
        # v with per-head ones column: [seq_part, jc, head, 64 v + 1 one]
        v_sb = const.tile([128, JC, NHL, HD + 1], F16, tag="v")
        nc.vector.memset(v_sb[:], 1.0)

        def emit_qk_proj(fc, which=None):
            """q^T,k^T projection for feature chunk fc (pack fc's 2 heads).
            which=0 emits only q, which=1 only k, None both."""
            parts = ((wq_sb, bq_sb, qT_sb), (wk_sb, bk_sb, kT_sb))
            if which is not None:
                parts = (parts[which],)
            for w_sb, b_sb, dst in parts:
                for sc in range(SC):
                    ps = ps_p.tile([128, 512], F32, tag="pp", name=f"pp{fc}{sc}")
                    for hc in range(HC):
                        nc.tensor.matmul(
                            ps[:],
                            w_sb[:, hc, fc * 128 : (fc + 1) * 128],
                            hsT_sb[:, hc, sc * 512 : (sc + 1) * 512],
                            start=(hc == 0),
                            stop=(hc == HC - 1),
                        )
                    nc.vector.tensor_scalar_add(
                        dst[:, fc, sc * 512 : (sc + 1) * 512],
                        ps[:],
                        b_sb[:, fc : fc + 1],
                    )

        def emit_v_proj():
            """v projection [seq, feat] + bias via ones-matmul."""
            for jc in range(JC):
                ps = ps_p.tile([128, 512], F32, tag="pp", name=f"ppv{jc}")
                for hc in range(HC):
                    nc.tensor.matmul(
                        ps[:],
                        hsT_sb[:, hc, jc * 128 : (jc + 1) * 128],
                        wv_sb[:, hc, :],
                        start=(hc == 0),
                        stop=False,
                    )
                nc.tensor.matmul(
                    ps[:], ones_sb[:, 0:128], bv_sb[:], start=False, stop=True
                )
                nc.vector.tensor_copy(
                    v_sb[:, jc, :, 0:HD], ps[:].rearrange("p (h d) -> p h d", h=NHL)
                )

        def emit_scores(g2, i, ptb):
            """Scores + exp for pack g2, query chunk i. The two heads are
            row-tiled concurrently on the PE (K=64 each) and share one N=1024
            ACT exp (same key-chunk -> same mask bias, exact for any mask)."""
            for jc in range(JC):
                ps = ps_s.tile([128, 1024], F32, tag="ss", name=f"ss{jc}")
                for hh in range(2):
                    lo = hh * 64
                    nc.tensor.matmul(
                        ps[:, hh * 512 : (hh + 1) * 512],
                        kT_sb[lo : lo + 64, g2, jc * 128 : (jc + 1) * 128],
                        qT_sb[lo : lo + 64, g2, i * 512 : (i + 1) * 512],
                        start=True,
                        stop=True,
                        tile_position=(lo, 0),
                    )
                nc.scalar.activation(
                    ptb[:, :, jc, :],
                    ps[:].rearrange("p (a b) -> p a b", a=2),
                    EXP,
                    bias=mb_sb[:, jc : jc + 1],
                    scale=0.125,
                )

        def emit_ctx(g2, i, ptb):
            """ctx~^T + normalize + store for pack g2, query chunk i."""
            for hh in range(2):
                h = 2 * g2 + hh
                pc = ps_c.tile([HD + 1, 512], F32, tag="cc", name=f"cc{hh}")
                for jc in range(JC):
                    nc.tensor.matmul(
                        pc[:],
                        v_sb[:, jc, h, :],
                        ptb[:, hh, jc, :],
                        start=(jc == 0),
                        stop=(jc == JC - 1),
                    )
                den = sm.tile([1, 512], F32, tag="den", name=f"dn{hh}")
                nc.vector.tensor_copy(den[:], pc[HD : HD + 1, :])
                recip = sm.tile([1, 512], F32, tag="recip", name=f"rc{hh}")
                nc.vector.reciprocal_approx_fast(recip[:], den[:])
                pbs = sm.tile([64, 512], F32, tag="pbs", name=f"pb{hh}")
                nc.gpsimd.partition_broadcast(pbs[:], recip[0:1, :])
                ob = sm.tile([64, 512], F32, tag="ob", name=f"ob{hh}")
                nc.vector.tensor_mul(ob[:], pc[0:HD, :], pbs[:])
                nc.sync.dma_start(out[h, :, i * 512 : (i + 1) * 512], ob[:])

        # ---- emission schedule: keep PE dense while ACT drains exp ----
        # pack-0 q/k projections up front so scores start as soon as the
        # first weight chunks land; each iteration then gets dependency-free
        # projection matmuls as PE filler while ACT chews this pack's exp:
        #   iter0: v-projection (ctx(0,0) needs it at iter end)
        #   iter1: pack-1 q+k;  iter2/3: pack-2 q/k;  iter4/5: pack-3 q/k
        with tc.tile_pool(name="ps_p0", bufs=4, space="PSUM") as ps_p0:
            qk0 = []
            for w_sb, b_sb, dst in ((wq_sb, bq_sb, qT_sb), (wk_sb, bk_sb, kT_sb)):
                for sc in range(SC):
                    ps = ps_p0.tile([128, 512], F32, tag="pp0", name=f"p0{sc}")
                    qk0.append((ps, w_sb, b_sb, dst, sc))
            for half in range(2):
                for ps, w_sb, b_sb, dst, sc in qk0:
                    for hc in range(half * HH, (half + 1) * HH):
                        nc.tensor.matmul(
                            ps[:],
                            w_sb[:, hc, 0:128],
                            hsT_sb[:, hc, sc * 512 : (sc + 1) * 512],
                            start=(hc == 0),
                            stop=(hc == HC - 1),
                        )
            for ps, w_sb, b_sb, dst, sc in qk0:
                nc.vector.tensor_scalar_add(
                    dst[:, 0, sc * 512 : (sc + 1) * 512], ps[:], b_sb[:, 0:1]
                )
        ps_s = ctx.enter_context(tc.tile_pool(name="ps_s", bufs=2, space="PSUM"))
        ps_c = ctx.enter_context(tc.tile_pool(name="ps_c", bufs=2, space="PSUM"))
        fillers = [
            emit_v_proj,
            lambda: emit_qk_proj(1),
            lambda: emit_qk_proj(2, which=0),
            lambda: emit_qk_proj(2, which=1),
            lambda: emit_qk_proj(3, which=0),
            lambda: emit_qk_proj(3, which=1),
            None,
            None,
        ]
        step = 0
        for g2 in range(4):
            for i in range(SC):
                ptb = p_pool.tile([128, 2, JC, 512], F16, tag="pt", name="ptb")
                emit_scores(g2, i, ptb)
                if fillers[step] is not None:
                    fillers[step]()
                step += 1
                emit_ctx(g2, i, ptb)

    nc.compile()
    return nc


_NC = None


def _get_nc():
    global _NC
    if _NC is None:
        _NC = _build_nc()
    return _NC


# test-harness knobs (ignored in normal grading use)
TRACE = False
TRACE_DIR = None
LAST_RESULT = None


def _pack(mT):
    """[1024, N] contraction-major -> [128, 8, N] partition-major fp16 so one
    DMA moves 8*N*2 contiguous bytes per partition (big DMA packets)."""
    n = mT.shape[1]
    return np.ascontiguousarray(
        mT.reshape(HC, 128, n).transpose(1, 0, 2)
    ).astype(np.float16)


def kernel(hidden_states, attention_mask, Wq, bq, Wk, bk, Wv, bv):
    global LAST_RESULT
    hs = np.asarray(hidden_states, dtype=np.float32)
    mask = np.asarray(attention_mask, dtype=np.float32)
    Wq = np.asarray(Wq, dtype=np.float32)
    Wk = np.asarray(Wk, dtype=np.float32)
    Wv = np.asarray(Wv, dtype=np.float32)
    bq = np.asarray(bq, dtype=np.float32)
    bk = np.asarray(bk, dtype=np.float32)
    bv = np.asarray(bv, dtype=np.float32)

    in_maps = []
    for c in range(NCORES):
        b, g = c % B, c // B
        sl = slice(g * FSH, (g + 1) * FSH)
        in_maps.append(
            {
                "hsT": _pack(hs[b].T),
                "wqT": _pack(Wq[sl, :].T),
                "wkT": _pack(Wk[sl, :].T),
                "wvT": _pack(Wv[sl, :].T),
                "bqc": np.ascontiguousarray(bq[sl].reshape(FC, 128).T),
                "bkc": np.ascontiguousarray(bk[sl].reshape(FC, 128).T),
                "bv1": np.ascontiguousarray(bv[sl].reshape(1, FSH)),
                "mb": np.ascontiguousarray(
                    ((mask[b, 0, 0, :] - 1.0) * 1.0e6).reshape(JC, 128).T
                ),
            }
        )

    nc = _get_nc()
    kw = {}
    if TRACE:
        kw = {"trace": True, "tmpdir": TRACE_DIR}
    res = run_bass_kernel_spmd(nc, in_maps, list(range(NCORES)), **kw)
    LAST_RESULT = res

    full = np.empty((B, S, HID), dtype=np.float32)
    for c in range(NCORES):
        b, g = c % B, c // B
        o = res.results[c]["out"]  # [NHL, HD, S]
        full[b, :, g * FSH : (g + 1) * FSH] = (
            o.transpose(2, 0, 1).reshape(S, FSH)
        )
    return full
